# revision 1
# baseline (speedup 1.0000x reference)
"""GAT encoder (3-layer) on 8 Trainium2 NeuronCores.

Sharding: nodes partitioned across cores (graph partition). Edges partitioned
by destination node so segment-softmax + scatter-add stay device-local.
Weights replicated. Per-layer halo exchange = AllGather of each core's node
feature shard (transposed layout).

Device algorithm per layer (per core, rank r owns nodes [r*6272,(r+1)*6272)):
  1. H table build (all 50176 nodes, redundant on every core, avoids a 2nd
     collective): psum = h^T_tile.T @ W -> HBM table rows [Wh(128)] (512B).
  2. alpha_d for own nodes: matvec W@a_dst against own h^T, broadcast to
     [128, NLOC+64] (cols NLOC.. = -1e9 sentinel for pad tokens), then
     GpSimd indirect_copy + SBUF reshape-DMA -> per-token alpha_d.
  3. Edge phase, chunks of 2048 tokens (host guarantees each chunk has
     UNIQUE dst indices -- HW scatter-add races RMW on duplicates):
       dma_gather 512B h rows by src (single_packet=False)
       alpha_s = reduce(h * a_src) on DVE
       p = exp(leakyrelu(a_s+a_d)); payload [p*h | p | junk] (192 f32)
       dma_scatter_add into alternating out_aug buffers [NLOC+2,192]
       (cross-chunk same-buffer WAW serialization makes dups safe; the
        alternating buffer keeps the DMA pipe full; row NLOC = pad scratch)
  4. Post: h = (sum p*h)/(sum p) + b, ELU; transpose -> h^T shard; AllGather.
  Final: global_mean_pool partial sums via one-hot matmul; host combines.
"""

import math
import numpy as np

# ---------------- constants (hardcoded problem shape) ----------------
N = 50000
F = 128
G = 64
NCORES = 8
NLOC = 6272                   # 49*128 nodes per core (padded)
NPAD = NLOC * NCORES          # 50176
NTILES = NLOC // 128          # 49
TTILES = NPAD // 128          # 392
ROW = 192                     # scatter payload row width (f32) -> 768B
RTAB = NPAD + 2               # table rows; 0 = padA, RTAB-1 = padB
BANK = 32768                  # gather bank split (int16 idx range)
CHUNK = 2048
C = CHUNK // 128              # 16 tokens per partition per chunk
IC_GROUP = 2                  # chunks per indirect-copy call (ISA dst limit 512)
NAUG = NLOC + 64              # alpha_d replicated width (sentinel tail)
NEG_SLOPE = 0.2
BIG_NEG = -1.0e9
EPS = 1.0e-16
OUTROWS = NLOC + 2            # scatter dst rows (row NLOC = pad scratch)
KBUF = 2                      # scatter accumulators per layer (WAW overlap)


# ---------------- host-side preprocessing ----------------

def _assign_chunks(gs, ld, nch):
    """Assign edges to chunks s.t. each chunk has unique dst (ld).
    Round-robin per dst, staggered by dst. Returns list of (gs, ld) arrays
    per chunk, or None if some chunk overflows CHUNK."""
    order = np.argsort(ld, kind="stable")
    gs_s, ld_s = gs[order], ld[order]
    # k-th edge of its dst group
    first = np.ones(len(ld_s), bool)
    first[1:] = ld_s[1:] != ld_s[:-1]
    gidx = np.cumsum(first) - 1
    starts = np.nonzero(first)[0]
    k = np.arange(len(ld_s)) - starts[gidx]
    ch = (ld_s + k) % nch
    chunks = []
    for ci in range(nch):
        m = ch == ci
        if m.sum() > CHUNK:
            return None
        chunks.append((gs_s[m], ld_s[m]))
    return chunks


def _ic_groups(nA, nB):
    groups = []
    for bank_start, n_b in ((0, nA), (nA, nB)):
        pos = 0
        while pos < n_b:
            sz = min(IC_GROUP, n_b - pos)
            groups.append((bank_start + pos, sz))
            pos += sz
    return groups


def _build_edge_data(src, dst):
    per_core = []
    for r in range(NCORES):
        lo, hi = r * NLOC, (r + 1) * NLOC
        m = (dst >= lo) & (dst < hi)
        gs = src[m].astype(np.int64) + 1          # physical table row
        ld = (dst[m] - lo).astype(np.int64)       # local dst
        mA = gs < BANK
        per_core.append(((gs[mA], ld[mA]), (gs[~mA] - BANK, ld[~mA])))

    def n_needed(pairs):
        n = 1
        for gs, ld in pairs:
            n = max(n, int(math.ceil(len(gs) / CHUNK)))
            if len(ld):
                n = max(n, int(np.bincount(ld).max()))
        return n

    nA = n_needed([a for a, _ in per_core])
    nB = n_needed([b for _, b in per_core])

    # chunk assignment (bump n on overflow)
    assigned = None
    while assigned is None:
        assigned = []
        for r in range(NCORES):
            (gA, lA), (gB, lB) = per_core[r]
            ca = _assign_chunks(gA, lA, nA)
            cb = _assign_chunks(gB, lB, nB)
            if ca is None:
                nA += 1
                assigned = None
                break
            if cb is None:
                nB += 1
                assigned = None
                break
            assigned.append(ca + cb)

    padA_idx, padB_idx = 0, RTAB - 1 - BANK
    nCH = nA + nB
    gidx = np.zeros((NCORES, nCH, 128, CHUNK // 16), np.int16)
    sidx = np.zeros((NCORES, nCH, 128, CHUNK // 16), np.int16)
    # gidx/sidx are concatenated into one [nCH, 128, 256] input later

    t = np.arange(CHUNK)
    tr, tc = t % 16, t // 16

    # aidx: big per-bank indirect-copy streams
    # bank token array: token tt -> (p = tt%128, col = tt//128)
    # group g stream pos i = k*C_all + j ; tt = j*128 + 16g + k
    def build_aidx(ld_tok, n):
        C_all = n * C
        M = 16 * C_all
        out = np.zeros((128, M // 16), np.uint16)
        i_arr = np.arange(M)
        k_arr = i_arr // C_all
        j_arr = i_arr % C_all
        rows = i_arr % 16
        cols = i_arr // 16
        for g in range(8):
            tt = j_arr * 128 + 16 * g + k_arr
            out[16 * g + rows, cols] = ld_tok[tt].astype(np.uint16)
        return out

    groups = _ic_groups(nA, nB)

    aidx_list = []
    for r in range(NCORES):
        chunks = assigned[r]
        ld_tok = np.full(nCH * CHUNK, NLOC, np.int64)
        for ci in range(nCH):
            gs_c, ld_c = chunks[ci]
            bankB = ci >= nA
            pad = padB_idx if bankB else padA_idx
            gfull = np.full(CHUNK, pad, np.int64)
            gfull[:len(gs_c)] = gs_c
            lfull = np.zeros(CHUNK, np.int64)
            lfull[:len(ld_c)] = ld_c
            lfull[len(ld_c):] = NLOC              # pad -> scratch row
            t16 = np.zeros((16, CHUNK // 16), np.int16)
            t16[tr, tc] = gfull.astype(np.int16)
            gidx[r, ci] = np.tile(t16, (8, 1))
            s16 = np.zeros((16, CHUNK // 16), np.int16)
            s16[tr, tc] = lfull.astype(np.int16)
            sidx[r, ci] = np.tile(s16, (8, 1))
            adl = lfull.copy()
            adl[len(ld_c):] = NLOC                # pad -> -1e9 sentinel
            ld_tok[ci * CHUNK:(ci + 1) * CHUNK] = adl
        parts = [build_aidx(ld_tok[c0 * CHUNK:(c0 + gsz) * CHUNK], gsz)
                 for c0, gsz in groups]
        aidx_list.append(np.concatenate(parts, axis=1))

    aidx = np.stack(aidx_list)                    # [NCORES, 128, nCH*C]
    return gidx, sidx, aidx, nA, nB, groups


def _prep_inputs(x, edge_index, batch, Ws, asrcs, adsts, bs):
    src = np.concatenate([edge_index[0], np.arange(N, dtype=np.int64)])
    dst = np.concatenate([edge_index[1], np.arange(N, dtype=np.int64)])
    src = np.asarray(src, np.int64)
    dst = np.asarray(dst, np.int64)

    gidx, sidx, aidx, nA, nB, groups = _build_edge_data(src, dst)

    xT_full = np.zeros((F, NPAD), np.float32)
    xT_full[:, :N] = np.asarray(x, np.float32).T

    w_aug = np.zeros((3, F, F + 1), np.float32)
    for k in range(3):
        w_aug[k, :, :F] = Ws[k]
        w_aug[k, :, F] = Ws[k] @ adsts[k]

    asrc_rep = np.zeros((3, 128, F), np.float32)
    b_rep = np.zeros((3, 128, F), np.float32)
    for k in range(3):
        asrc_rep[k] = np.tile(asrcs[k][None, :], (128, 1))
        b_rep[k] = np.tile(bs[k][None, :], (128, 1))

    zrow = np.zeros((OUTROWS, ROW), np.float32)

    batch64 = np.asarray(batch, np.int64)
    phot = np.zeros((NCORES, NTILES, 128, G), np.float32)
    for r in range(NCORES):
        base = r * NLOC
        for j in range(NTILES):
            nodes = base + j * 128 + np.arange(128)
            valid = nodes < N
            gsel = batch64[np.minimum(nodes, N - 1)]
            ph = np.zeros((128, G), np.float32)
            ph[np.arange(128)[valid], gsel[valid]] = 1.0
            phot[r, j] = ph

    counts = np.bincount(batch64, minlength=G).astype(np.float32)

    in_maps = []
    for r in range(NCORES):
        in_maps.append({
            "xT_full": xT_full,
            "xT_own": np.ascontiguousarray(xT_full[:, r * NLOC:(r + 1) * NLOC]),
            "w_aug": w_aug,
            "asrc_rep": asrc_rep,
            "b_rep": b_rep,
            "zrow": zrow,
            "gsidx": np.concatenate([gidx[r], sidx[r]], axis=2),
            "aidx": aidx[r],
            "phot": phot[r].reshape(NTILES * 128, G),
        })
    return in_maps, nA, nB, counts


# ---------------- numpy emulation of the device program ----------------

def _emulate_full(in_maps, nA, nB, counts):
    nCH = nA + nB
    hT_cur = [im["xT_own"].copy() for im in in_maps]
    hT_ag = None
    pool_part = [np.zeros((G, F), np.float32) for _ in range(NCORES)]
    for k in range(3):
        new_hT = []
        for r in range(NCORES):
            im = in_maps[r]
            w = im["w_aug"][k]
            a_src = im["asrc_rep"][k][0]
            table = np.zeros((RTAB, F), np.float32)
            hsrc = im["xT_full"] if k == 0 else hT_ag
            table[1:1 + NPAD] = (hsrc.T @ w[:, :F]).astype(np.float32)
            ad_aug = np.full(NAUG, BIG_NEG, np.float32)
            ad_aug[:NLOC] = (w[:, F][None, :] @ hT_cur[r])[0]
            out_aug = np.zeros((OUTROWS, ROW), np.float32)
            for ci in range(nCH):
                bank_base = 0 if ci < nA else BANK
                g16 = im["gsidx"][ci, :, :CHUNK // 16].astype(np.int64)
                s16 = im["gsidx"][ci, :, CHUNK // 16:].astype(np.int64)
                t = np.arange(CHUNK)
                gtok = g16[t % 16, t // 16]
                stok = s16[t % 16, t // 16]
                gbuf = table[bank_base + gtok]                 # [CHUNK,128]
                # alpha_d via grouped indirect copy emulation
                groups = _ic_groups(nA, nB)
                for c0, gsz in groups:
                    if c0 <= ci < c0 + gsz:
                        break
                C_all = gsz * C
                a16 = im["aidx"][:, c0 * C:(c0 + gsz) * C].astype(np.int64)
                base_col = (ci - c0) * C
                ad_tok = np.zeros(CHUNK, np.float32)
                for g in range(8):
                    iarr = np.arange(16 * C_all)
                    stream = a16[16 * g + iarr % 16, iarr // 16]
                    kk = iarr // C_all
                    jj = iarr % C_all
                    sel = (jj >= base_col) & (jj < base_col + C)
                    tt_local = (jj[sel] - base_col) * 128 + 16 * g + kk[sel]
                    ad_tok[tt_local] = ad_aug[stream[sel]]
                al_s = gbuf @ a_src
                e = (al_s + ad_tok).astype(np.float32)
                e = np.maximum(e, NEG_SLOPE * e)
                p = np.exp(e).astype(np.float32)
                payload = np.zeros((CHUNK, ROW), np.float32)
                payload[:, :F] = gbuf * p[:, None]
                payload[:, F] = p
                np.add.at(out_aug, stok, payload)
            s = out_aug[:NLOC, F] + EPS
            h1 = (out_aug[:NLOC, :F] / s[:, None]
                  + im["b_rep"][k][0][None, :]).astype(np.float32)
            hout = np.where(h1 > 0, h1,
                            np.exp(np.minimum(h1, 0)) - 1).astype(np.float32)
            if k < 2:
                new_hT.append(hout.T.copy())
            else:
                ph = im["phot"].reshape(NTILES, 128, G)
                for j in range(NTILES):
                    pool_part[r] += ph[j].T @ hout[128 * j:128 * j + 128]
        if k < 2:
            hT_ag = np.concatenate(new_hT, axis=1)
            hT_cur = new_hT
    total = np.sum(pool_part, axis=0)
    return (total / np.maximum(counts, 1.0)[:, None]).astype(np.float32)


# ---------------- bass program ----------------

def _build_program(nA, nB, features=("gather", "ic", "scatter", "cc"),
                   repeat=1):
    import concourse.bacc as bacc
    import concourse.bass as bass
    import concourse.mybir as mybir
    import concourse.tile as tile
    from concourse import masks
    features = set(features)

    f32 = mybir.dt.float32
    i16 = mybir.dt.int16
    u16 = mybir.dt.uint16
    AF = mybir.ActivationFunctionType
    ALU = mybir.AluOpType
    AX = mybir.AxisListType
    nCH = nA + nB
    MA_COLS = nA * C          # aidx cols for bank A (per 16 rows)
    MB_COLS = nB * C

    nc = bacc.Bacc("TRN2", target_bir_lowering=False, debug=False,
                   num_devices=NCORES)

    # --- dram I/O ---
    xT_full = nc.dram_tensor("xT_full", [F, NPAD], f32, kind="ExternalInput")
    xT_own = nc.dram_tensor("xT_own", [F, NLOC], f32, kind="ExternalInput")
    w_aug_d = nc.dram_tensor("w_aug", [3, F, F + 1], f32, kind="ExternalInput")
    asrc_d = nc.dram_tensor("asrc_rep", [3, 128, F], f32, kind="ExternalInput")
    b_rep_d = nc.dram_tensor("b_rep", [3, 128, F], f32, kind="ExternalInput")
    zrow_d = nc.dram_tensor("zrow", [OUTROWS, ROW], f32, kind="ExternalInput")
    gsidx_d = nc.dram_tensor("gsidx", [nCH, 128, 2 * (CHUNK // 16)], i16,
                             kind="ExternalInput")
    aidx_d = nc.dram_tensor("aidx", [128, MA_COLS + MB_COLS], u16,
                            kind="ExternalInput")
    phot_d = nc.dram_tensor("phot", [NTILES * 128, G], f32,
                            kind="ExternalInput")
    pool_out = nc.dram_tensor("pool_part", [G, F], f32, kind="ExternalOutput")

    # --- internal dram ---
    h_table = nc.dram_tensor("h_table", [RTAB, F], f32, kind="Internal")
    out_augs = [nc.dram_tensor(f"out_aug{i}", [OUTROWS, ROW], f32,
                               kind="Internal") for i in range(3 * KBUF)]
    cc_in = nc.dram_tensor("cc_in", [F, NLOC], f32, kind="Internal")
    cc_out = nc.dram_tensor("cc_out", [NCORES, F, NLOC], f32, kind="Internal",
                            addr_space="Shared")

    with tile.TileContext(nc) as tc:
        with (
            tc.tile_pool(name="persist", bufs=1) as persist,
            tc.tile_pool(name="lhs", bufs=4) as lhs_pool,
            tc.tile_pool(name="stage", bufs=4) as stage_pool,
            tc.tile_pool(name="edge", bufs=3) as edge_pool,
            tc.tile_pool(name="gb", bufs=2) as gb_pool,
            tc.tile_pool(name="post", bufs=3) as post_pool,
            tc.tile_pool(name="ps", bufs=2, space="PSUM") as ps_pool,
            tc.tile_pool(name="pstr", bufs=2, space="PSUM") as pstr_pool,
            tc.tile_pool(name="ps1", bufs=1, space="PSUM") as ps1_pool,
            tc.tile_pool(name="psb", bufs=1, space="PSUM") as psb_pool,
            tc.tile_pool(name="pspool", bufs=1, space="PSUM") as pspool_pool,
        ):
            # persistent tiles
            hT = persist.tile([F, NLOC], f32, tag="hT")
            ad_rep = persist.tile([128, NAUG], f32, tag="ad_rep")
            ad_row = persist.tile([1, NLOC], f32, tag="ad_row")
            adt_all = persist.tile([128, nCH * C], f32, tag="adt_all")
            identity = persist.tile([128, 128], f32, tag="identity")
            ones_col = persist.tile([1, 128], f32, tag="ones_col")
            w_sb = persist.tile([F, F + 1], f32, tag="w_sb")
            asrc_sb = persist.tile([128, F], f32, tag="asrc_sb")
            b_sb = persist.tile([128, F], f32, tag="b_sb")
            ic_out = persist.tile([128, 16 * IC_GROUP * C], f32,
                                  tag="ic_out")
            pay_bufs = [persist.tile([128, C, ROW], f32, tag=f"pay{i}",
                                     name=f"pay{i}")
                        for i in range(KBUF)]
            aidx_sb = persist.tile([128, MA_COLS + MB_COLS], u16,
                                   tag="aidx_sb")

            masks.make_identity(nc, identity[:])
            nc.gpsimd.memset(ones_col[:], 1.0)
            nc.sync.dma_start(aidx_sb[:], aidx_d.ap())
            # zero pad rows of the gather table
            zpad = persist.tile([2, F], f32, tag="zpad")
            nc.gpsimd.memset(zpad[:], 0.0)
            nc.sync.dma_start(h_table.ap()[0:1], zpad[0:1])
            nc.sync.dma_start(h_table.ap()[RTAB - 1:RTAB], zpad[1:2])

            for pb_ in pay_bufs:
                nc.vector.memset(pb_[:, :, F + 1:ROW], 0.0)
            for rep in range(repeat):
              nc.sync.dma_start(hT[:], xT_own.ap())
              for oa in out_augs:
                nc.scalar.dma_start(oa.ap()[:], zrow_d.ap()[:])
              for k in range(3):
                  nc.sync.dma_start(w_sb[:], w_aug_d.ap()[k])
                  nc.sync.dma_start(asrc_sb[:], asrc_d.ap()[k])
                  nc.sync.dma_start(b_sb[:], b_rep_d.ap()[k])

                  # ---- table build: all NPAD nodes, blocked ----
                  # block loads/stores cut HWDGE instruction count; loads on
                  # SP queue, stores on ACT queue to parallelize sequencers
                  def table_block(t0, nt, load_src):
                      lhsT = lhs_pool.tile([128, 4, 128], f32, tag="lhsT")
                      load_src(lhsT, t0, nt)
                      ps = ps_pool.tile([128, 4, F], f32, tag="ps_tab")
                      for i in range(nt):
                          nc.tensor.matmul(ps[:, i], lhsT[:, i], w_sb[:, 0:F],
                                           start=True, stop=True)
                      st = stage_pool.tile([128, 4, F], f32, tag="stage")
                      nc.scalar.activation(st[:, 0:nt], ps[:, 0:nt], AF.Copy)
                      dst = h_table.ap()[1 + 128 * t0:1 + 128 * (t0 + nt)] \
                          .rearrange("(t p) f -> p t f", t=nt)
                      nc.scalar.dma_start(dst, st[:, 0:nt])

                  if "notable" in features:
                      pass
                  elif k == 0:
                      def load0(lhsT, t0, nt):
                          nc.sync.dma_start(
                              lhsT[:, 0:nt],
                              xT_full.ap()[:, 128 * t0:128 * (t0 + nt)]
                              .rearrange("p (t f) -> p t f", t=nt))
                      for blk in range(TTILES // 4):
                          table_block(4 * blk, 4, load0)
                  else:
                      def load1(lhsT, t0, nt):
                          rr, jj = t0 // NTILES, t0 % NTILES
                          nc.sync.dma_start(
                              lhsT[:, 0:nt],
                              cc_out.ap()[rr, :, 128 * jj:128 * (jj + nt)]
                              .rearrange("p (t f) -> p t f", t=nt))
                      for rr in range(NCORES):
                          base = rr * NTILES
                          pos = 0
                          while pos < NTILES:
                              nt = min(4, NTILES - pos)
                              table_block(base + pos, nt, load1)
                              pos += nt

                  # ---- alpha_d of own nodes -> replicated [128, NAUG] ----
                  ad_chunks = []
                  pos = 0
                  while pos < NLOC:
                      sz = min(512, NLOC - pos)
                      ad_chunks.append((pos, sz))
                      pos += sz
                  for pos, sz in ad_chunks:
                      sl = slice(pos, pos + sz)
                      pr = ps1_pool.tile([1, 512], f32, tag="ps_ad")
                      nc.tensor.matmul(pr[:, 0:sz], w_sb[:, F:F + 1],
                                       hT[:, sl], start=True, stop=True)
                      nc.vector.tensor_copy(ad_row[:, sl], pr[:, 0:sz])
                  for pos, sz in ad_chunks:
                      sl = slice(pos, pos + sz)
                      pb = psb_pool.tile([128, 512], f32, tag="ps_bc")
                      nc.tensor.matmul(pb[:, 0:sz], ones_col[:], ad_row[:, sl],
                                       start=True, stop=True)
                      nc.vector.tensor_copy(ad_rep[:, sl], pb[:, 0:sz])
                  nc.vector.memset(ad_rep[:, NLOC:NAUG], BIG_NEG)

                  # ---- per-token alpha_d: 2 big indirect copies + reshape ----
                  if "ic" in features:
                      for c0, gsz in _ic_groups(nA, nB):
                          C_all = gsz * C
                          M = 16 * C_all
                          nc.gpsimd.indirect_copy(
                              ic_out[:, 0:M], ad_rep[:],
                              aidx_sb[:, c0 * C:c0 * C + C_all], True)
                          src_ap = ic_out[:, 0:M].rearrange(
                              "(g o) (kk j) -> g o kk j",
                              g=8, o=16, kk=16, j=C_all)[:, 0]
                          nc.sync.dma_start(
                              adt_all[:, c0 * C:c0 * C + C_all], src_ap)
                  else:
                      nc.vector.memset(adt_all[:], 0.0)

                  # ---- edge phase ----
                  for ci in range(nCH):
                      bank = h_table.ap()[0:BANK] if ci < nA \
                          else h_table.ap()[BANK:RTAB]
                      gsi = edge_pool.tile([128, 2 * (CHUNK // 16)], i16,
                                           tag="gsi")
                      nc.sync.dma_start(gsi[:], gsidx_d.ap()[ci])
                      gi = gsi[:, 0:CHUNK // 16]
                      si = gsi[:, CHUNK // 16:2 * (CHUNK // 16)]

                      gbuf = gb_pool.tile([128, C, F], f32, tag="gbuf")
                      if "gather" in features:
                          nc.gpsimd.dma_gather(gbuf[:], bank, gi, CHUNK,
                                               CHUNK, F, single_packet=False)
                      else:
                          nc.vector.memset(gbuf[:], 0.0)

                      als = edge_pool.tile([128, C], f32, tag="als")
                      if "noedve" in features:
                          nc.vector.memset(als[:], 0.0)
                      else:
                          prod = edge_pool.tile([128, C, F], f32, tag="prod")
                          a_bc = asrc_sb[:].unsqueeze(1).broadcast_to([128, C, F])
                          nc.vector.tensor_tensor(prod[:], gbuf[:], a_bc,
                                                  ALU.mult)
                          nc.vector.tensor_reduce(als[:], prod[:], AX.X, ALU.add)

                      e = edge_pool.tile([128, C], f32, tag="e")
                      nc.vector.tensor_tensor(e[:], als[:],
                                              adt_all[:, ci * C:ci * C + C],
                                              ALU.add)
                      nc.vector.scalar_tensor_tensor(e[:], e[:], NEG_SLOPE, e[:],
                                                     ALU.mult, ALU.max)
                      p = edge_pool.tile([128, C], f32, tag="p")
                      nc.scalar.activation(p[:], e[:], AF.Exp)

                      pay = pay_bufs[ci % KBUF]
                      if "noedve" not in features:
                          p_b = p[:].unsqueeze(2).broadcast_to([128, C, F])
                          nc.vector.tensor_tensor(pay[:, :, 0:F], gbuf[:], p_b,
                                                  ALU.mult)
                          nc.vector.tensor_copy(pay[:, :, F], p[:])
                      if "scatter" in features:
                          nc.gpsimd.dma_scatter_add(
                              out_augs[2 * k + ci % 2].ap()[:], pay[:], si,
                              CHUNK, CHUNK, ROW, single_packet=False)

                  # ---- post-process ----
                  if k == 2:
                      ps_pl = pspool_pool.tile([G, F], f32, tag="ps_pl")
                  for j in range(NTILES):
                      poA = post_pool.tile([128, F + 1], f32, tag="poA")
                      poB = post_pool.tile([128, F + 1], f32, tag="poB")
                      nc.scalar.dma_start(
                          poA[:],
                          out_augs[2 * k].ap()[128 * j:128 * j + 128, 0:F + 1])
                      nc.scalar.dma_start(
                          poB[:],
                          out_augs[2 * k + 1].ap()[128 * j:128 * j + 128, 0:F + 1])
                      po = post_pool.tile([128, F + 1], f32, tag="po")
                      nc.vector.tensor_tensor(po[:], poA[:], poB[:], ALU.add)
                      s_t = post_pool.tile([128, 1], f32, tag="s_t")
                      nc.vector.tensor_scalar_add(s_t[:], po[:, F:F + 1], EPS)
                      r_t = post_pool.tile([128, 1], f32, tag="r_t")
                      nc.vector.reciprocal(r_t[:], s_t[:])
                      h1 = post_pool.tile([128, F], f32, tag="h1")
                      nc.vector.tensor_scalar(h1[:], po[:, 0:F], r_t[:], None,
                                              ALU.mult)
                      nc.vector.tensor_tensor(h1[:], h1[:], b_sb[:], ALU.add)
                      # ELU = relu(x) + expm1(min(x,0))
                      mn = post_pool.tile([128, F], f32, tag="mn")
                      nc.vector.tensor_scalar_min(mn[:], h1[:], 0.0)
                      ex = post_pool.tile([128, F], f32, tag="ex")
                      nc.scalar.activation(ex[:], mn[:], AF.Exp)
                      rl = post_pool.tile([128, F], f32, tag="rl")
                      nc.vector.tensor_scalar_max(rl[:], h1[:], 0.0)
                      ho = post_pool.tile([128, F], f32, tag="ho")
                      nc.vector.scalar_tensor_tensor(ho[:], ex[:], -1.0, rl[:],
                                                     ALU.add, ALU.add)
                      if k < 2:
                          pt = pstr_pool.tile([128, 128], f32, tag="ps_tr")
                          nc.tensor.transpose(pt[:], ho[:], identity[:])
                          nc.vector.tensor_copy(hT[:, 128 * j:128 * j + 128],
                                                pt[:])
                      else:
                          ph = post_pool.tile([128, G], f32, tag="ph")
                          nc.sync.dma_start(
                              ph[:], phot_d.ap()[128 * j:128 * j + 128])
                          nc.tensor.matmul(ps_pl[:], ph[:], ho[:],
                                           start=(j == 0),
                                           stop=(j == NTILES - 1))

                  if k < 2:
                      nc.sync.dma_start(cc_in.ap()[:], hT[:])
                      if "cc" in features:
                          nc.gpsimd.collective_compute(
                              "AllGather", mybir.AluOpType.bypass,
                              replica_groups=[list(range(NCORES))],
                              ins=[cc_in.ap().opt()], outs=[cc_out.ap().opt()])
                      else:
                          for rr in range(NCORES):
                              nc.sync.dma_start(cc_out.ap()[rr], cc_in.ap()[:])
                  else:
                      pl_sb = post_pool.tile([G, F], f32, tag="pl_sb")
                      nc.vector.tensor_copy(pl_sb[:], ps_pl[:])
                      nc.sync.dma_start(pool_out.ap()[:], pl_sb[:])

    nc.compile()
    return nc


# ---------------- entry point ----------------

LAST_EXEC_NS = None


def kernel(x, edge_index, batch,
           W1, a_src1, a_dst1, b1,
           W2, a_src2, a_dst2, b2,
           W3, a_src3, a_dst3, b3):
    global LAST_EXEC_NS
    x = np.asarray(x, np.float32)
    edge_index = np.asarray(edge_index)
    batch = np.asarray(batch)
    Ws = [np.asarray(W1, np.float32), np.asarray(W2, np.float32),
          np.asarray(W3, np.float32)]
    asrcs = [np.asarray(a_src1, np.float32), np.asarray(a_src2, np.float32),
             np.asarray(a_src3, np.float32)]
    adsts = [np.asarray(a_dst1, np.float32), np.asarray(a_dst2, np.float32),
             np.asarray(a_dst3, np.float32)]
    bs = [np.asarray(b1, np.float32), np.asarray(b2, np.float32),
          np.asarray(b3, np.float32)]

    in_maps, nA, nB, counts = _prep_inputs(x, edge_index, batch, Ws, asrcs,
                                           adsts, bs)

    from concourse.bass_utils import run_bass_kernel_spmd
    nc = _build_program(nA, nB)
    res = run_bass_kernel_spmd(nc, in_maps, core_ids=list(range(NCORES)))
    LAST_EXEC_NS = res.exec_time_ns
    total = np.zeros((G, F), np.float32)
    for r in range(NCORES):
        total += res.results[r]["pool_part"]
    out = total / np.maximum(counts, 1.0)[:, None]
    return out.astype(np.float32)



# revision 31
# speedup vs baseline: 2.6795x; 2.6795x over previous
"""GAT encoder (3-layer) on 8 Trainium2 NeuronCores — scatter-free design.

Sharding: nodes partitioned across cores (graph partition). Edges partitioned
by destination node; weights replicated.

Key design vs the earlier gather+scatter version: the HW profile showed the
GpSimd Q7 core serially generating DMA descriptors for dma_gather AND
dma_scatter_add (~30us per 2048-edge chunk) while the DMA engines idled at
~50%. This version removes the scatter entirely and shrinks the table build:

  1. Per layer, each core computes Wh for ITS OWN nodes only (49 matmuls)
     and the AllGather of those [NLOC, F] bf16 shards IS the gather table
     (node-major [NPAD, F] in shared DRAM). No redundant 392-tile build,
     no separate table store, bf16 rows halve gather bytes.
  2. Edges are grouped by destination TILE (128 consecutive dst nodes), one
     chunk per tile. Segment softmax + scatter-add happen ON-CHIP: a DVE
     iota-compare builds S'[token, seg] = p_token one-hot-weighted, and
     16 PE matmuls accumulate out[seg, :] += S'_g.T @ gbuf_g in PSUM
     (fp32 accumulate). Sum-of-p comes from S'_g.T @ ones. Post-processing
     (divide, bias, ELU, transpose/pool) runs per chunk from PSUM — the
     out_aug HBM round-trip is gone.
  3. Gather idx streams carry trailing -1 pads (trimmed by Q7 before
     descriptor gen, so per-core count variance costs nothing); in-stream
     pads are killed by segid=-1 (S' row = 0) and alpha_d sentinel -1e9.
  4. Gathers cycle over 4 SWDGE queues so one chunk's drain overlaps the
     next chunk's descriptor generation.
"""

import math
import numpy as np

# ---------------- constants (hardcoded problem shape) ----------------
N = 50000
F = 128
G = 64
NCORES = 8
NLOC = 6272                   # 49*128 nodes per core (padded)
NPAD = NLOC * NCORES          # 50176
NTILES = NLOC // 128          # 49 = dst tiles per core = chunks per layer
BANK = 32768                  # gather bank split (int16 idx range)
NAUG = NLOC + 64              # alpha_d replicated width (sentinel tail)
NEG_SLOPE = 0.2
BIG_NEG = -1.0e9
EPS = 1.0e-16
IC_GROUP = 2                  # chunks per indirect-copy call (ISA dst limit 512)
PADFILL = True                # pad gather idx streams with 0 to full width


# ---------------- host-side preprocessing ----------------

def _build_edge_data(src, dst):
    """Group edges by (core, dst-tile, src-bank); build per-chunk gather idx
    streams (wrapped-16, trailing -1 pads), segid arrays, and the alpha_d
    indirect-copy idx stream."""
    per = {}
    for r in range(NCORES):
        lo, hi = r * NLOC, (r + 1) * NLOC
        m = (dst >= lo) & (dst < hi)
        gs = src[m].astype(np.int64)
        ld = (dst[m] - lo).astype(np.int64)
        tile = ld // 128
        seg = ld % 128
        bankB = gs >= BANK
        for j in range(NTILES):
            tm = tile == j
            mA = tm & ~bankB
            mB = tm & bankB
            per[(r, j)] = (gs[mA], seg[mA], gs[mB] - BANK, seg[mB])

    # per-chunk bank widths (max over cores, rounded to 16)
    CA = np.zeros(NTILES, np.int64)
    CB = np.zeros(NTILES, np.int64)
    for j in range(NTILES):
        for r in range(NCORES):
            gA, _, gB, _ = per[(r, j)]
            CA[j] = max(CA[j], len(gA))
            CB[j] = max(CB[j], len(gB))
    CA = ((CA + 15) // 16) * 16
    CB = ((CB + 15) // 16) * 16
    CACOLS = (CA + 127) // 128
    CBCOLS = (CB + 127) // 128
    if PADFILL:
        CA = CACOLS * 128
        CB = CBCOLS * 128
    COLS = CACOLS + CBCOLS                      # slot cols per chunk
    OFF = np.concatenate([[0], np.cumsum(COLS)])  # cumulative col offsets
    GSW = (CA + CB) // 16                        # idx words per chunk
    GSOFF = np.concatenate([[0], np.cumsum(GSW)])

    def wrap16(vals, width):
        """vals (int) -> [16, width//16] wrapped (token t -> [t%16, t//16]),
        then tiled to [128, width//16]."""
        a = np.full(width, 0 if PADFILL else -1, np.int64)
        a[:len(vals)] = vals
        t = np.arange(width)
        w = np.zeros((16, width // 16), np.int16)
        w[t % 16, t // 16] = a.astype(np.int16)
        return np.tile(w, (8, 1))

    # adt IC stream is padded to a fixed ICC=16 cols (2048 slots) per chunk
    # so the grouped indirect copies keep the known-good 512-elem geometry.
    ICC = 16
    assert COLS.max() <= ICC
    gsi = np.zeros((NCORES, 128, GSOFF[-1]), np.int16)
    segid = np.full((NCORES, 128, OFF[-1]), -1.0, np.float32)
    ld_tok = np.full((NCORES, NTILES * ICC * 128), NLOC, np.int64)

    for r in range(NCORES):
        for j in range(NTILES):
            gA, sA, gB, sB = per[(r, j)]
            gsi[r, :, GSOFF[j]:GSOFF[j] + CA[j] // 16] = wrap16(gA, CA[j])
            gsi[r, :, GSOFF[j] + CA[j] // 16:GSOFF[j + 1]] = wrap16(gB, CB[j])
            # slot s (= col*128 + p) -> seg / local dst
            base = j * ICC * 128
            for (g, s, s0) in ((gA, sA, 0), (gB, sB, 128 * CACOLS[j])):
                tt = s0 + np.arange(len(g))
                segid[r, tt % 128, OFF[j] + tt // 128] = s.astype(np.float32)
                ld_tok[r, base + tt] = j * 128 + s
    return dict(per=per, CA=CA, CB=CB, CACOLS=CACOLS, CBCOLS=CBCOLS,
                COLS=COLS, OFF=OFF, GSW=GSW, GSOFF=GSOFF,
                ICC=ICC), gsi, segid, ld_tok


def _ic_groups(n_chunks, icc, ic_limit=512):
    """Group consecutive chunks for the alpha_d indirect copy; each group
    covers gsz chunks of icc cols each, 16*gsz*icc <= ic_limit."""
    per = ic_limit // (16 * icc)
    groups = []
    pos = 0
    while pos < n_chunks:
        sz = min(per, n_chunks - pos)
        groups.append((pos, sz, sz * icc))
        pos += sz
    return groups


def _build_aidx(ld_tok_r, meta, groups):
    """Build the u16 idx stream for the grouped indirect copies.
    Group covering chunks [c0, c0+gsz) has C_all = gsz*ICC columns;
    IC output stream position i on partition 16*gg + (i%16), col i//16 maps
    to token tt = j*128 + 16*gg + k where k = i//C_all, j = i%C_all
    (each 16-partition group gg covers tokens with residue [16gg, 16gg+16))."""
    icc = meta["ICC"]
    parts = []
    for (c0, gsz, C_all) in groups:
        M = 16 * C_all
        out = np.zeros((128, M // 16), np.uint16)
        i_arr = np.arange(M)
        k_arr = i_arr // C_all
        j_arr = i_arr % C_all
        base = c0 * icc * 128
        ld = ld_tok_r[base:base + 128 * C_all]
        for gg in range(8):
            tt = j_arr * 128 + 16 * gg + k_arr
            out[16 * gg + i_arr % 16, i_arr // 16] = ld[tt].astype(np.uint16)
        parts.append(out)
    return np.concatenate(parts, axis=1)


def _prep_inputs(x, edge_index, batch, Ws, asrcs, adsts, bs):
    src = np.concatenate([edge_index[0], np.arange(N, dtype=np.int64)])
    dst = np.concatenate([edge_index[1], np.arange(N, dtype=np.int64)])
    src = np.asarray(src, np.int64)
    dst = np.asarray(dst, np.int64)

    meta, gsi, segid, ld_tok = _build_edge_data(src, dst)
    groups = _ic_groups(NTILES, meta["ICC"])
    meta["groups"] = groups

    xT_own = np.zeros((NCORES, F, NLOC), np.float32)
    xf = np.asarray(x, np.float32).T
    for r in range(NCORES):
        lo = r * NLOC
        w = min(NLOC, max(0, N - lo))
        xT_own[r, :, :w] = xf[:, lo:lo + w]

    w_aug = np.zeros((3, F, F + 1), np.float32)
    for k in range(3):
        w_aug[k, :, :F] = Ws[k]
        w_aug[k, :, F] = Ws[k] @ adsts[k]

    asrc_rep = np.zeros((3, 128, F), np.float32)
    b_rep = np.zeros((3, 128, F), np.float32)
    for k in range(3):
        asrc_rep[k] = np.tile(asrcs[k][None, :], (128, 1))
        b_rep[k] = np.tile(bs[k][None, :], (128, 1))

    iota = np.tile(np.arange(128, dtype=np.float32)[None, :], (128, 1))

    batch64 = np.asarray(batch, np.int64)
    phot = np.zeros((NCORES, NTILES, 128, G), np.float32)
    for r in range(NCORES):
        base = r * NLOC
        for j in range(NTILES):
            nodes = base + j * 128 + np.arange(128)
            valid = nodes < N
            gsel = batch64[np.minimum(nodes, N - 1)]
            ph = np.zeros((128, G), np.float32)
            ph[np.arange(128)[valid], gsel[valid]] = 1.0
            phot[r, j] = ph

    counts = np.bincount(batch64, minlength=G).astype(np.float32)

    in_maps = []
    for r in range(NCORES):
        in_maps.append({
            "xT_own": np.ascontiguousarray(xT_own[r]),
            "w_aug": w_aug,
            "asrc_rep": asrc_rep,
            "b_rep": b_rep,
            "gsi": np.ascontiguousarray(gsi[r]),
            "segid": np.ascontiguousarray(segid[r]),
            "aidx": _build_aidx(ld_tok[r], meta, groups),
            "iota": iota,
            "phot": phot[r].reshape(NTILES * 128, G),
        })
    return in_maps, meta, counts


# ---------------- numpy emulation of the device program ----------------

def _emulate_full(in_maps, meta, counts):
    CA, CB = meta["CA"], meta["CB"]
    CACOLS, COLS, OFF, GSOFF = meta["CACOLS"], meta["COLS"], meta["OFF"], meta["GSOFF"]
    hT_cur = [im["xT_own"].copy() for im in in_maps]
    pool_part = [np.zeros((G, F), np.float32) for _ in range(NCORES)]
    for k in range(3):
        # table = allgather of own Wh
        tabs = []
        ad_reps = []
        for r in range(NCORES):
            w = in_maps[r]["w_aug"][k]
            tabs.append((hT_cur[r].T @ w[:, :F]).astype(np.float32))
            ad = (w[:, F][None, :] @ hT_cur[r])[0]
            ad_aug = np.full(NAUG, BIG_NEG, np.float32)
            ad_aug[:NLOC] = ad
            ad_reps.append(ad_aug)
        table = np.concatenate(tabs, axis=0)          # [NPAD, F]
        new_hT = []
        for r in range(NCORES):
            im = in_maps[r]
            a_src = im["asrc_rep"][k][0]
            b = im["b_rep"][k][0]
            # adt via the aidx emulation (validates _build_aidx)
            ICC = meta["ICC"]
            adt_all = np.zeros((128, NTILES * ICC), np.float32)
            aoff = 0
            for (c0, gsz, C_all) in meta["groups"]:
                M = 16 * C_all
                i_arr = np.arange(M)
                k_arr = i_arr // C_all
                j_arr = i_arr % C_all
                a16 = im["aidx"][:, aoff:aoff + C_all].astype(np.int64)
                aoff += C_all
                for gg in range(8):
                    stream = a16[16 * gg + i_arr % 16, i_arr // 16]
                    vals = ad_reps[r][stream]
                    # stream pos i -> token (j*128 + 16gg + k) of group slots
                    tt = j_arr * 128 + 16 * gg + k_arr
                    adt_all[tt % 128, ICC * c0 + tt // 128] = vals
            ho_all = np.zeros((NLOC, F), np.float32)
            for j in range(NTILES):
                nslots = COLS[j] * 128
                gbuf = np.zeros((nslots, F), np.float32)
                gs = im["gsi"][:16, GSOFF[j]:GSOFF[j + 1]]
                tA = np.arange(CA[j])
                idxA = gs[tA % 16, tA // 16].astype(np.int64)
                vA = idxA >= 0
                gbuf[tA[vA]] = table[idxA[vA]]
                tB = np.arange(CB[j])
                idxB = gs[tB % 16, CA[j] // 16 + tB // 16].astype(np.int64)
                vB = idxB >= 0
                gbuf[128 * CACOLS[j] + tB[vB]] = table[BANK + idxB[vB]]
                als = gbuf @ a_src
                sg = im["segid"][:, OFF[j]:OFF[j + 1]]
                t = np.arange(nslots)
                seg_t = sg[t % 128, t // 128]
                ad_t = adt_all[t % 128, ICC * j + t // 128]
                e = als + ad_t
                e = np.maximum(e, NEG_SLOPE * e)
                p = np.exp(e).astype(np.float32)
                mask = seg_t[:, None] == np.arange(128)[None, :]
                sp = mask * p[:, None]                 # [slots, 128seg]
                out = sp.T @ gbuf                      # [128, F]
                ssum = sp.T @ np.ones(nslots, np.float32)
                h1 = out / (ssum[:, None] + EPS) + b[None, :]
                ho = np.where(h1 > 0, h1, np.exp(np.minimum(h1, 0)) - 1)
                ho_all[128 * j:128 * (j + 1)] = ho.astype(np.float32)
            if k < 2:
                new_hT.append(ho_all.T.copy())
            else:
                ph = im["phot"].reshape(NTILES, 128, G)
                for j in range(NTILES):
                    pool_part[r] += ph[j].T @ ho_all[128 * j:128 * (j + 1)]
        if k < 2:
            hT_cur = new_hT
    total = np.sum(pool_part, axis=0)
    return (total / np.maximum(counts, 1.0)[:, None]).astype(np.float32)


# ---------------- bass program ----------------

def _build_program(meta, repeat=1, nqueues=4, features=("gather", "ic", "cc", "mm")):
    import concourse.bacc as bacc
    import concourse.bass as bass
    import concourse.mybir as mybir
    import concourse.tile as tile
    from concourse import masks

    f32 = mybir.dt.float32
    bf16 = mybir.dt.bfloat16
    i16 = mybir.dt.int16
    u16 = mybir.dt.uint16
    AF = mybir.ActivationFunctionType
    ALU = mybir.AluOpType
    AX = mybir.AxisListType

    CA = [int(v) for v in meta["CA"]]
    CB = [int(v) for v in meta["CB"]]
    CACOLS = [int(v) for v in meta["CACOLS"]]
    COLS = [int(v) for v in meta["COLS"]]
    OFF = [int(v) for v in meta["OFF"]]
    GSW = [int(v) for v in meta["GSW"]]
    GSOFF = [int(v) for v in meta["GSOFF"]]
    features = set(features)
    groups = meta["groups"]
    ICC = meta["ICC"]
    TOTCOLS = OFF[-1]
    GSTOT = GSOFF[-1]
    AW = sum(g[2] for g in groups)
    CMAX = max(COLS)

    nc = bacc.Bacc("TRN2", target_bir_lowering=False, debug=False,
                   num_devices=NCORES, num_swdge_queues=nqueues)

    # --- dram I/O ---
    xT_own = nc.dram_tensor("xT_own", [F, NLOC], f32, kind="ExternalInput")
    w_aug_d = nc.dram_tensor("w_aug", [3, F, F + 1], f32, kind="ExternalInput")
    asrc_d = nc.dram_tensor("asrc_rep", [3, 128, F], f32, kind="ExternalInput")
    b_rep_d = nc.dram_tensor("b_rep", [3, 128, F], f32, kind="ExternalInput")
    gsi_d = nc.dram_tensor("gsi", [128, GSTOT], i16, kind="ExternalInput")
    segid_d = nc.dram_tensor("segid", [128, TOTCOLS], f32, kind="ExternalInput")
    aidx_d = nc.dram_tensor("aidx", [128, AW], u16, kind="ExternalInput")
    iota_d = nc.dram_tensor("iota", [128, 128], f32, kind="ExternalInput")
    phot_d = nc.dram_tensor("phot", [NTILES * 128, G], f32,
                            kind="ExternalInput")
    pool_out = nc.dram_tensor("pool_part", [G, F], f32, kind="ExternalOutput")

    tdt = f32 if "f32tab" in features else bf16
    # --- internal dram ---
    cc_in = nc.dram_tensor("cc_in", [NLOC, F], tdt, kind="Internal")
    cc_out = nc.dram_tensor("cc_out", [NCORES, NLOC, F], tdt, kind="Internal",
                            addr_space="Shared")
    if "tabcopy" in features:
        tab_int = nc.dram_tensor("tab_int", [NPAD, F], tdt, kind="Internal")
        table = tab_int.ap()
    else:
        table = cc_out.ap().rearrange("r n f -> (r n) f")

    with tile.TileContext(nc) as tc:
        with (
            tc.tile_pool(name="persist", bufs=1) as persist,
            tc.tile_pool(name="lhs", bufs=3) as lhs_pool,
            tc.tile_pool(name="stage", bufs=3) as stage_pool,
            tc.tile_pool(name="gb", bufs=3) as gb_pool,
            tc.tile_pool(name="sp", bufs=3) as sp_pool,
            tc.tile_pool(name="edge", bufs=3) as edge_pool,
            tc.tile_pool(name="post", bufs=3) as post_pool,
            tc.tile_pool(name="pst", bufs=2, space="PSUM") as pst_pool,
            tc.tile_pool(name="ptr", bufs=1, space="PSUM") as ptr_pool,
            tc.tile_pool(name="ps1", bufs=1, space="PSUM") as ps1_pool,
            tc.tile_pool(name="psb", bufs=1, space="PSUM") as psb_pool,
            tc.tile_pool(name="pso", bufs=2, space="PSUM") as pso_pool,
            tc.tile_pool(name="pspool", bufs=1, space="PSUM") as pspool_pool,
        ):
            # persistent tiles
            hT = persist.tile([F, NLOC], f32, tag="hT")
            ad_rep = persist.tile([128, NAUG], f32, tag="ad_rep")
            ad_row = persist.tile([1, NLOC], f32, tag="ad_row")
            adt_all = persist.tile([128, NTILES * ICC], f32, tag="adt_all")
            identity = persist.tile([128, 128], f32, tag="identity")
            ones_col = persist.tile([1, 128], f32, tag="ones_col")
            onesb = persist.tile([128, 1], tdt, tag="onesb")
            w_sb = persist.tile([F, F + 1], f32, tag="w_sb")
            asrc_bf = persist.tile([128, F], tdt, tag="asrc_bf")
            asrc_sb = persist.tile([128, F], f32, tag="asrc_sb")
            b_sb = persist.tile([128, F], f32, tag="b_sb")
            ic_out = persist.tile([128, 512], f32, tag="ic_out")
            aidx_sb = persist.tile([128, AW], u16, tag="aidx_sb")
            gsi_sb = persist.tile([128, GSTOT], i16, tag="gsi_sb")
            segid_sb = persist.tile([128, TOTCOLS], f32, tag="segid_sb")
            iota_sb = persist.tile([128, 128], f32, tag="iota_sb")

            masks.make_identity(nc, identity[:])
            nc.gpsimd.memset(ones_col[:], 1.0)
            nc.vector.memset(onesb[:], 1.0)
            nc.sync.dma_start(aidx_sb[:], aidx_d.ap())
            nc.sync.dma_start(gsi_sb[:], gsi_d.ap())
            nc.sync.dma_start(segid_sb[:], segid_d.ap())
            nc.sync.dma_start(iota_sb[:], iota_d.ap())

            for rep in range(repeat):
              nc.sync.dma_start(hT[:], xT_own.ap())
              for k in range(3):
                  nc.sync.dma_start(w_sb[:], w_aug_d.ap()[k])
                  nc.sync.dma_start(asrc_sb[:], asrc_d.ap()[k])
                  nc.sync.dma_start(b_sb[:], b_rep_d.ap()[k])
                  nc.vector.tensor_copy(asrc_bf[:], asrc_sb[:])

                  # ---- own-shard Wh -> cc_in (bf16), then AllGather = table
                  pos = 0
                  while pos < NTILES:
                      nt = min(4, NTILES - pos)
                      ps = pst_pool.tile([128, 4, F], f32, tag="ps_tab")
                      for i in range(nt):
                          nc.tensor.matmul(
                              ps[:, i], hT[:, 128 * (pos + i):128 * (pos + i + 1)],
                              w_sb[:, 0:F], start=True, stop=True)
                      st = stage_pool.tile([128, 4, F], tdt, tag="stage")
                      nc.scalar.activation(st[:, 0:nt], ps[:, 0:nt], AF.Copy)
                      dst = cc_in.ap()[128 * pos:128 * (pos + nt)] \
                          .rearrange("(t p) f -> p t f", t=nt)
                      nc.scalar.dma_start(dst, st[:, 0:nt])
                      pos += nt
                  if "cc" in features:
                      nc.gpsimd.collective_compute(
                          "AllGather", mybir.AluOpType.bypass,
                          replica_groups=[list(range(NCORES))],
                          ins=[cc_in.ap().opt()], outs=[cc_out.ap().opt()])
                  else:
                      for rr in range(NCORES):
                          nc.sync.dma_start(cc_out.ap()[rr], cc_in.ap()[:])
                  if "tabcopy" in features:
                      for rr in range(NCORES):
                          nc.scalar.dma_start(
                              tab_int.ap()[rr * NLOC:(rr + 1) * NLOC],
                              cc_out.ap()[rr])

                  # ---- alpha_d of own nodes -> replicated [128, NAUG] ----
                  ad_chunks = []
                  pos = 0
                  while pos < NLOC:
                      sz = min(512, NLOC - pos)
                      ad_chunks.append((pos, sz))
                      pos += sz
                  for pos, sz in ad_chunks:
                      sl = slice(pos, pos + sz)
                      pr = ps1_pool.tile([1, 512], f32, tag="ps_ad")
                      nc.tensor.matmul(pr[:, 0:sz], w_sb[:, F:F + 1],
                                       hT[:, sl], start=True, stop=True)
                      nc.vector.tensor_copy(ad_row[:, sl], pr[:, 0:sz])
                  for pos, sz in ad_chunks:
                      sl = slice(pos, pos + sz)
                      pb = psb_pool.tile([128, 512], f32, tag="ps_bc")
                      nc.tensor.matmul(pb[:, 0:sz], ones_col[:], ad_row[:, sl],
                                       start=True, stop=True)
                      nc.vector.tensor_copy(ad_rep[:, sl], pb[:, 0:sz])
                  nc.vector.memset(ad_rep[:, NLOC:NAUG], BIG_NEG)

                  # ---- per-token alpha_d via grouped indirect copies ----
                  if "ic" in features:
                      aoff = 0
                      for (c0, gsz, C_all) in groups:
                          M = 16 * C_all
                          nc.gpsimd.indirect_copy(
                              ic_out[:, 0:M], ad_rep[:],
                              aidx_sb[:, aoff:aoff + C_all], True)
                          src_ap = ic_out[:, 0:M].rearrange(
                              "(g o) (kk j) -> g o kk j",
                              g=8, o=16, kk=16, j=C_all)[:, 0]
                          nc.sync.dma_start(
                              adt_all[:, ICC * c0:ICC * c0 + C_all], src_ap)
                          aoff += C_all
                  else:
                      nc.vector.memset(adt_all[:], 0.0)

                  # ---- edge phase: one chunk per dst tile ----
                  if k == 2:
                      ps_pl = pspool_pool.tile([G, F], f32, tag="ps_pl")
                  for j in range(NTILES):
                      cj = COLS[j]
                      gbuf = gb_pool.tile([128, CMAX, F], tdt, tag="gbuf")
                      if "gather" not in features:
                          nc.vector.memset(gbuf[:], 0.0)
                      else:
                          if CA[j] > 0:
                              nc.gpsimd.dma_gather(
                                  gbuf[:, 0:CACOLS[j]], table[0:BANK],
                                  gsi_sb[:, GSOFF[j]:GSOFF[j] + CA[j] // 16],
                                  CA[j], CA[j], F, single_packet=False,
                                  queue_num=(2 * j) % nqueues)
                          if CB[j] > 0:
                              nc.gpsimd.dma_gather(
                                  gbuf[:, CACOLS[j]:cj], table[BANK:NPAD],
                                  gsi_sb[:, GSOFF[j] + CA[j] // 16:GSOFF[j + 1]],
                                  CB[j], CB[j], F, single_packet=False,
                                  queue_num=(2 * j + 1) % nqueues)

                      # als = reduce(gbuf * a_src)
                      prod = edge_pool.tile([128, CMAX, F], tdt, tag="prod")
                      a_bc = asrc_bf[:].unsqueeze(1).broadcast_to([128, cj, F])
                      nc.vector.tensor_tensor(prod[:, 0:cj], gbuf[:, 0:cj],
                                              a_bc, ALU.mult)
                      als = edge_pool.tile([128, CMAX], f32, tag="als")
                      nc.vector.tensor_reduce(als[:, 0:cj], prod[:, 0:cj],
                                              AX.X, ALU.add)
                      e = edge_pool.tile([128, CMAX], f32, tag="e")
                      nc.vector.tensor_tensor(e[:, 0:cj], als[:, 0:cj],
                                              adt_all[:, ICC * j:ICC * j + cj],
                                              ALU.add)
                      nc.vector.scalar_tensor_tensor(
                          e[:, 0:cj], e[:, 0:cj], NEG_SLOPE, e[:, 0:cj],
                          ALU.mult, ALU.max)
                      p_bf = edge_pool.tile([128, CMAX], tdt, tag="p_bf")
                      nc.scalar.activation(p_bf[:, 0:cj], e[:, 0:cj], AF.Exp)

                      # S'[token, seg] = p * (segid == seg)
                      sp = sp_pool.tile([128, CMAX, 128], tdt, tag="sp")
                      sg_b = segid_sb[:, OFF[j]:OFF[j + 1]] \
                          .unsqueeze(2).broadcast_to([128, cj, 128])
                      io_b = iota_sb[:].unsqueeze(1).broadcast_to([128, cj, 128])
                      nc.vector.tensor_tensor(sp[:, 0:cj], sg_b, io_b,
                                              ALU.is_equal)
                      p_b = p_bf[:, 0:cj].unsqueeze(2).broadcast_to([128, cj, 128])
                      nc.vector.tensor_tensor(sp[:, 0:cj], sp[:, 0:cj], p_b,
                                              ALU.mult)

                      # segment sums in PSUM: out[seg, 0:F] and sum_p at F
                      ps_o = pso_pool.tile([128, F + 4], f32, tag="ps_o")
                      if "mm" in features:
                          for g in range(cj):
                              nc.tensor.matmul(ps_o[:, 0:F], sp[:, g], gbuf[:, g],
                                               start=(g == 0), stop=(g == cj - 1))
                          for g in range(cj):
                              nc.tensor.matmul(ps_o[:, F:F + 1], sp[:, g], onesb[:],
                                               start=(g == 0), stop=(g == cj - 1))
                      else:
                          nc.tensor.matmul(ps_o[:, 0:F], sp[:, 0], gbuf[:, 0],
                                           start=True, stop=True)
                          nc.tensor.matmul(ps_o[:, F:F + 1], sp[:, 0], onesb[:],
                                           start=True, stop=True)

                      # ---- post: h = out/sum_p + b, ELU ----
                      s_t = post_pool.tile([128, 1], f32, tag="s_t")
                      nc.vector.tensor_scalar_add(s_t[:], ps_o[:, F:F + 1], EPS)
                      r_t = post_pool.tile([128, 1], f32, tag="r_t")
                      nc.vector.reciprocal(r_t[:], s_t[:])
                      h1 = post_pool.tile([128, F], f32, tag="h1")
                      nc.vector.tensor_scalar(h1[:], ps_o[:, 0:F], r_t[:], None,
                                              ALU.mult)
                      nc.vector.tensor_tensor(h1[:], h1[:], b_sb[:], ALU.add)
                      mn = post_pool.tile([128, F], f32, tag="mn")
                      nc.vector.tensor_scalar_min(mn[:], h1[:], 0.0)
                      ex = post_pool.tile([128, F], f32, tag="ex")
                      nc.scalar.activation(ex[:], mn[:], AF.Exp)
                      rl = post_pool.tile([128, F], f32, tag="rl")
                      nc.vector.tensor_scalar_max(rl[:], h1[:], 0.0)
                      ho = post_pool.tile([128, F], f32, tag="ho")
                      nc.vector.scalar_tensor_tensor(ho[:], ex[:], -1.0, rl[:],
                                                     ALU.add, ALU.add)
                      if k < 2:
                          pt = ptr_pool.tile([128, 128], f32, tag="ps_tr")
                          nc.tensor.transpose(pt[:], ho[:], identity[:])
                          nc.vector.tensor_copy(hT[:, 128 * j:128 * (j + 1)],
                                                pt[:])
                      else:
                          ph = post_pool.tile([128, G], f32, tag="ph")
                          nc.sync.dma_start(
                              ph[:], phot_d.ap()[128 * j:128 * (j + 1)])
                          nc.tensor.matmul(ps_pl[:], ph[:], ho[:],
                                           start=(j == 0),
                                           stop=(j == NTILES - 1))

                  if k == 2:
                      pl_sb = post_pool.tile([G, F], f32, tag="pl_sb")
                      nc.vector.tensor_copy(pl_sb[:], ps_pl[:])
                      nc.sync.dma_start(pool_out.ap()[:], pl_sb[:])

    nc.compile()
    return nc


# ---------------- entry point ----------------

LAST_EXEC_NS = None
LAST_META = None


def kernel(x, edge_index, batch,
           W1, a_src1, a_dst1, b1,
           W2, a_src2, a_dst2, b2,
           W3, a_src3, a_dst3, b3):
    global LAST_EXEC_NS, LAST_META
    x = np.asarray(x, np.float32)
    edge_index = np.asarray(edge_index)
    batch = np.asarray(batch)
    Ws = [np.asarray(W1, np.float32), np.asarray(W2, np.float32),
          np.asarray(W3, np.float32)]
    asrcs = [np.asarray(a_src1, np.float32), np.asarray(a_src2, np.float32),
             np.asarray(a_src3, np.float32)]
    adsts = [np.asarray(a_dst1, np.float32), np.asarray(a_dst2, np.float32),
             np.asarray(a_dst3, np.float32)]
    bs = [np.asarray(b1, np.float32), np.asarray(b2, np.float32),
          np.asarray(b3, np.float32)]

    in_maps, meta, counts = _prep_inputs(x, edge_index, batch, Ws, asrcs,
                                         adsts, bs)
    LAST_META = meta

    from concourse.bass_utils import run_bass_kernel_spmd
    nc = _build_program(meta)
    res = run_bass_kernel_spmd(nc, in_maps, core_ids=list(range(NCORES)))
    LAST_EXEC_NS = res.exec_time_ns
    total = np.zeros((G, F), np.float32)
    for r in range(NCORES):
        total += res.results[r]["pool_part"]
    out = total / np.maximum(counts, 1.0)[:, None]
    return out.astype(np.float32)
